# revision 24
# baseline (speedup 1.0000x reference)
"""Trainium2 Bass kernel for nn_MultiHeadAttention (GQA + RoPE + causal softmax).

Problem (hardcoded): B=4, T=2048, C=2048, n_head=16, n_kv_head=4, head_dim=128,
fp32 in/out, rope base 10000, torch-Linear style projections (x @ W.T).

Sharding: 8 cores = (4 batches) x (2 head-halves). Each core handles one batch
and 8 consecutive query heads (= 2 full KV groups) over the full sequence.
q/k/v are column-parallel, o_proj row-parallel; the host sums the two partial
outputs per batch (no device collectives). All cores run an identical
instruction stream (SPMD) with identical causal structure - no parity waste.

All matmuls run in float32r (TF32-like). Everything on device is laid out
transposed ([feature, token]); the host pre-transposes x and the weights.
Q and Y live entirely in SBUF (no DRAM scratch round-trips). RoPE's
rotate-half is a PE permutation matmul (R constant) instead of SBUF-to-SBUF
DMAs. Softmax denominator: ones[128x128] matmul accumulating over key chunks
produces the denominator replicated across all partitions, so the reciprocal
multiplies directly (no broadcast matmul).
"""

import sys
import math

sys.path.insert(0, "/opt/trn_rl_repo")

import numpy as np
import ml_dtypes

import concourse.bacc as bacc
import concourse.mybir as mybir
import concourse.tile as tile
from concourse.bass_utils import run_bass_kernel_spmd

F32 = mybir.dt.float32
F32R = mybir.dt.float32r
BF16 = mybir.dt.bfloat16
AF = mybir.ActivationFunctionType

B, T, C = 4, 2048, 2048
NH, NKV, HD = 16, 4, 128
ROPE_BASE = 10000.0
NHL = NH // 2                 # 8 local query heads per core
NKVL = NKV // 2               # 2 local kv groups per core
CL = NHL * HD                 # 1024 local head dims
NCC = C // 128                # 16 contraction chunks
NTB = T // 256                # 8 token blocks (KV stage)
NJJ = T // 256                # 8 query pair-blocks (attention)
NQTR = T // 512               # 4 query quarters (Q stage)
MASK_NEG = -30000.0


def _build_nc(nrep=1):
    nc = bacc.Bacc(trn_type="TRN2", name="mha_gqa_rope_tp")

    xT = nc.dram_tensor("xT", [C, T], BF16, kind="ExternalInput")
    wqT = nc.dram_tensor("wqT", [C, CL], BF16, kind="ExternalInput")
    wkT = nc.dram_tensor("wkT", [C, NKVL * HD], BF16, kind="ExternalInput")
    wvT = nc.dram_tensor("wvT", [C, NKVL * HD], BF16, kind="ExternalInput")
    woT = nc.dram_tensor("woT", [CL, C], BF16, kind="ExternalInput")
    cosT = nc.dram_tensor("cosT", [HD, T], BF16, kind="ExternalInput")
    sinT = nc.dram_tensor("sinT", [HD, T], BF16, kind="ExternalInput")
    maskadd = nc.dram_tensor("maskadd", [128, 1024], BF16, kind="ExternalInput")
    consts = nc.dram_tensor("consts", [128, 256], F32R, kind="ExternalInput")
    ones_bf = nc.dram_tensor("ones_bf", [128, 128], BF16, kind="ExternalInput")
    outT = nc.dram_tensor("outT", [C, T], F32, kind="ExternalOutput")

    with tile.TileContext(nc) as tc:
        with tc.tile_pool(name="const", bufs=1) as constp, \
             tc.tile_pool(name="trig", bufs=1) as trigp:
            const_s = constp.tile([128, 256], F32R)
            rswap_l = const_s[:, 128:256]  # rotate-half permutation
            ones_s = constp.tile([128, 128], BF16)

            cos_s = trigp.tile([HD, T], BF16)
            sin_s = trigp.tile([HD, T], BF16)
            mask_s = constp.tile([128, 1024], BF16)

            for _rep in range(nrep):
                with tc.tile_pool(name="kv_res", bufs=1) as kvres, \
                     tc.tile_pool(name="q_res", bufs=1) as qres:
                    kT_s = kvres.tile([128, NKVL, T], BF16)          # [d, g, t]
                    v_s = kvres.tile([128, T // 128, NKVL * HD], BF16)  # [t%128, tc, vd]
                    q_sbuf = qres.tile([128, NHL, T], BF16)          # [d, h, t]

                    # ------- wq pool spans Stage KV (prefetch) + Stage Q ----
                    with tc.tile_pool(name="wq", bufs=1) as wqp, \
                         tc.tile_pool(name="xtile", bufs=4) as xtp, \
                         tc.tile_pool(name="qpsum", bufs=2, space="PSUM") as qps, \
                         tc.tile_pool(name="qrpsum", bufs=2, space="PSUM") as qrps:
                        wq_all = wqp.tile([128, NHL, NCC, 128], BF16)

                        # ---------------- Stage KV: K^T (roped) and V ------
                        with tc.tile_pool(name="wkv", bufs=1) as wkvp, \
                             tc.tile_pool(name="krope", bufs=3) as krp, \
                             tc.tile_pool(name="kpsum", bufs=1, space="PSUM") as kps, \
                             tc.tile_pool(name="krpsum", bufs=1, space="PSUM") as krps, \
                             tc.tile_pool(name="vpsum", bufs=2, space="PSUM") as vps:
                            wk_s = wkvp.tile([128, NCC, NKVL * HD], BF16, tag="wk")
                            nc.sync.dma_start(
                                out=wk_s[:], in_=wkT.ap().rearrange("(c p) k -> p c k", p=128)
                            )
                            # first x block in two halves so matmuls start early
                            xt0 = xtp.tile([128, NCC, 256], BF16, tag="xt")
                            for xh in range(2):
                                nc.sync.dma_start(
                                    out=xt0[:, xh * 8:(xh + 1) * 8],
                                    in_=xT.ap()[xh * 1024:(xh + 1) * 1024, 0:256].rearrange(
                                        "(c p) t -> p c t", p=128
                                    ),
                                )
                            if _rep == 0:
                                nc.sync.dma_start(out=const_s[:], in_=consts.ap())
                                nc.sync.dma_start(out=ones_s[:], in_=ones_bf.ap())
                            wv_s = wkvp.tile([128, NCC, NKVL * HD], BF16, tag="wv")
                            nc.sync.dma_start(
                                out=wv_s[:], in_=wvT.ap().rearrange("(c p) k -> p c k", p=128)
                            )
                            # prefetch xt(tb=1) ahead of the constant loads
                            xt1 = xtp.tile([128, NCC, 256], BF16, tag="xt")
                            nc.sync.dma_start(
                                out=xt1[:],
                                in_=xT.ap()[:, 256:512].rearrange("(c p) t -> p c t", p=128),
                            )
                            if _rep == 0:
                                nc.sync.dma_start(out=cos_s[:], in_=cosT.ap())
                                nc.sync.dma_start(out=sin_s[:], in_=sinT.ap())
                                nc.sync.dma_start(out=mask_s[:], in_=maskadd.ap())
                            for tb in range(NTB):
                                sl = slice(tb * 256, (tb + 1) * 256)
                                if tb == 0:
                                    xt = xt0
                                elif tb == 1:
                                    xt = xt1
                                else:
                                    xt = xtp.tile([128, NCC, 256], BF16, tag="xt")
                                    nc.sync.dma_start(
                                        out=xt[:],
                                        in_=xT.ap()[:, sl].rearrange("(c p) t -> p c t", p=128),
                                    )
                                # spread Wq strip prefetch over the KV stage
                                # (after tb=2 so early xt loads go first)
                                if 2 <= tb <= 5:
                                    for qs in (2 * tb - 4, 2 * tb - 3):
                                        nc.sync.dma_start(
                                            out=wq_all[:, qs],
                                            in_=wqT.ap()[:, qs * 128:(qs + 1) * 128].rearrange(
                                                "(c p) m -> p c m", p=128
                                            ),
                                        )
                                psk = kps.tile([128, NKVL, 256], F32, tag="psk")
                                for g in range(NKVL):
                                    for c in range(NCC):
                                        nc.tensor.matmul(
                                            psk[:, g, :],
                                            wk_s[:, c, g * 128:(g + 1) * 128],
                                            xt[:, c, :],
                                            start=(c == 0),
                                            stop=(c == NCC - 1),
                                        )
                                k0 = krp.tile([128, NKVL, 256], F32R, tag="k0")
                                nc.scalar.copy(k0[:], psk[:])
                                # V matmuls fill PE while ACT copies k0
                                psvs = []
                                for ti in range(2):
                                    psv = vps.tile([128, NKVL * HD], F32, tag="psv")
                                    for c in range(NCC):
                                        nc.tensor.matmul(
                                            psv[:],
                                            xt[:, c, ti * 128:(ti + 1) * 128],
                                            wv_s[:, c, :],
                                            start=(c == 0),
                                            stop=(c == NCC - 1),
                                        )
                                    psvs.append(psv)
                                rotk = krps.tile([128, NKVL * 256], F32, tag="rotk")
                                nc.tensor.matmul(
                                    rotk[:], rswap_l,
                                    k0[:].rearrange("p g t -> p (g t)"),
                                    start=True, stop=True,
                                )
                                for ti in range(2):
                                    nc.scalar.copy(v_s[:, tb * 2 + ti, :], psvs[ti])
                                rotk_g = rotk[:].rearrange("p (g t) -> p g t", g=NKVL)
                                t1 = krp.tile([128, NKVL, 256], F32, tag="kt1")
                                t2 = krp.tile([128, NKVL, 256], F32, tag="kt2")
                                for g in range(NKVL):
                                    nc.vector.tensor_mul(t1[:, g, :], k0[:, g, :], cos_s[:, sl])
                                    nc.vector.tensor_mul(t2[:, g, :], rotk_g[:, g, :], sin_s[:, sl])
                                    nc.vector.tensor_add(kT_s[:, g, sl], t1[:, g, :], t2[:, g, :])

                        # ---------------- Stage Q: Q^T = rope(WqT.T @ xT) --
                        with tc.tile_pool(name="qrope", bufs=2) as qrp:
                            pend = None  # (q0, sl, qc) rope tail, one head behind

                            def flush_rope():
                                nonlocal pend
                                if pend is None:
                                    return
                                q0p, slp, qcp = pend
                                rotq = qrps.tile([128, 256], F32, tag="rotq")
                                nc.tensor.matmul(
                                    rotq[:], rswap_l, q0p[:], start=True, stop=True
                                )
                                t1 = qrp.tile([128, 256], F32, tag="qt1")
                                t2 = qrp.tile([128, 256], F32, tag="qt2")
                                nc.vector.tensor_mul(t1[:], q0p[:], cos_s[:, slp])
                                nc.vector.tensor_mul(t2[:], rotq[:], sin_s[:, slp])
                                nc.vector.tensor_add(q_sbuf[:, qcp, slp], t1[:], t2[:])
                                pend = None

                            for qb in range(T // 256):
                                sl = slice(qb * 256, (qb + 1) * 256)
                                xq = xtp.tile([128, NCC, 256], BF16, tag="xt")
                                nc.sync.dma_start(
                                    out=xq[:],
                                    in_=xT.ap()[:, sl].rearrange("(c p) t -> p c t", p=128),
                                )
                                for qc in range(NHL):
                                    psq = qps.tile([128, 256], F32, tag="psq")
                                    for c in range(NCC):
                                        nc.tensor.matmul(
                                            psq[:],
                                            wq_all[:, qc, c, :],
                                            xq[:, c, :],
                                            start=(c == 0),
                                            stop=(c == NCC - 1),
                                        )
                                    q0 = qrp.tile([128, 256], F32R, tag="q0")
                                    nc.scalar.copy(q0[:], psq[:])
                                    flush_rope()
                                    pend = (q0, sl, qc)
                            flush_rope()

                    # ---------------- Stage C: attention -------------------
                    with tc.tile_pool(name="y_res", bufs=1) as yres, \
                         tc.tile_pool(name="wo", bufs=3) as wop, \
                         tc.tile_pool(name="oout", bufs=3) as ooutp:
                        y_sbuf = yres.tile([128, NHL, T], BF16)   # [d, h, t]
                        # prefetch first Wo strips on the idle Pool queue
                        wo_tiles = {}
                        for oc in range(2):
                            w = wop.tile([128, NHL, 128], BF16, tag="wo")
                            nc.gpsimd.dma_start(
                                out=w[:],
                                in_=woT.ap()[:, oc * 128:(oc + 1) * 128].rearrange(
                                    "(c p) m -> p c m", p=128
                                ),
                            )
                            wo_tiles[oc] = w
                        with tc.tile_pool(name="ptile", bufs=4) as ppp, \
                             tc.tile_pool(name="rtile", bufs=2) as rpp, \
                             tc.tile_pool(name="spsum", bufs=4, space="PSUM") as sps, \
                             tc.tile_pool(name="opsum", bufs=2, space="PSUM") as ops, \
                             tc.tile_pool(name="dpsum", bufs=2, space="PSUM") as dps:
                            pending_norm = None  # (den, po, hh, qsl), one pass behind

                            def flush_norm():
                                nonlocal pending_norm
                                if pending_norm is None:
                                    return
                                denp, pop, hhp, qslp = pending_norm
                                rec = rpp.tile([128, 512], F32R, tag="rec")
                                with nc.allow_low_precision(reason="f32r softmax recip"):
                                    nc.vector.reciprocal(rec[:], denp[:])
                                nc.vector.tensor_mul(
                                    y_sbuf[:, hhp:hhp + 2, qslp],
                                    pop[:].rearrange("p (h q) -> p h q", h=2),
                                    rec[:].rearrange("p (h q) -> p h q", h=2),
                                )
                                pending_norm = None

                            for jj in range(NJJ):
                                qsl = slice(jj * 256, (jj + 1) * 256)
                                nchunks = 2 * jj + 2
                                for gl in range(NKVL):
                                    for hp in range(2):       # two heads per pass
                                        hh = gl * 4 + hp * 2  # local head index
                                        den = dps.tile([128, 512], F32, tag="den")
                                        po = ops.tile([128, 512], F32, tag="po")
                                        queue = []  # exp'd chunks awaiting den/po

                                        def drain_one():
                                            ptq, ccq, diag = queue.pop(0)
                                            if diag:
                                                # only sub1 halves live on the last chunk
                                                ptv = ptq[:].rearrange(
                                                    "p (h q) -> p h q", h=2)[:, :, 128:256]
                                                denv = den[:].rearrange(
                                                    "p (h q) -> p h q", h=2)[:, :, 128:256]
                                                pov = po[:].rearrange(
                                                    "p (h q) -> p h q", h=2)[:, :, 128:256]
                                            else:
                                                ptv, denv, pov = ptq[:], den[:], po[:]
                                            nc.tensor.matmul(
                                                denv,
                                                ones_s[:],
                                                ptv,
                                                start=(ccq == 0),
                                                stop=(ccq == nchunks - 1),
                                            )
                                            nc.tensor.matmul(
                                                pov,
                                                v_s[:, ccq, gl * 128:(gl + 1) * 128],
                                                ptv,
                                                start=(ccq == 0),
                                                stop=(ccq == nchunks - 1),
                                            )

                                        for cc in range(nchunks):
                                            cb = cc - 2 * jj
                                            diag = cb == 1
                                            pss = sps.tile([128, 512], F32, tag="pss")
                                            pt = ppp.tile([128, 512], BF16, tag="pt")
                                            if diag:
                                                pssv = pss[:].rearrange(
                                                    "p (h q) -> p h q", h=2)[:, :, 128:256]
                                                qv = q_sbuf[
                                                    :, hh:hh + 2,
                                                    jj * 256 + 128:jj * 256 + 256,
                                                ]
                                                maskv = mask_s[:].rearrange(
                                                    "p (b h q) -> p b h q", b=2, h=2
                                                )[:, 1, :, 128:256]
                                                ptv = pt[:].rearrange(
                                                    "p (h q) -> p h q", h=2)[:, :, 128:256]
                                            else:
                                                pssv = pss[:]
                                                qv = q_sbuf[:, hh:hh + 2, qsl]
                                                maskv = mask_s[:, 0:512] if cb == 0 else None
                                                ptv = pt[:]
                                            nc.tensor.matmul(
                                                pssv,
                                                kT_s[:, gl, cc * 128:(cc + 1) * 128],
                                                qv,
                                                start=True,
                                                stop=True,
                                            )
                                            if cb >= 0:
                                                nc.vector.tensor_add(pssv, pssv, maskv)
                                            nc.scalar.activation(ptv, pssv, AF.Exp)
                                            queue.append((pt, cc, diag))
                                            if len(queue) > 2:
                                                drain_one()
                                        while queue:
                                            drain_one()
                                        flush_norm()
                                        pending_norm = (den, po, hh, qsl)
                            flush_norm()

                        # ------------ Stage D: partial out^T = WoT.T @ y^T --
                        with tc.tile_pool(name="opsum2", bufs=3, space="PSUM") as ops2:
                            for oc in range(NCC):
                                if oc in wo_tiles:
                                    wo_strip = wo_tiles[oc]
                                else:
                                    wo_strip = wop.tile([128, NHL, 128], BF16, tag="wo")
                                    nc.sync.dma_start(
                                        out=wo_strip[:],
                                        in_=woT.ap()[:, oc * 128:(oc + 1) * 128].rearrange(
                                            "(c p) m -> p c m", p=128
                                        ),
                                    )
                                for rbh in range(2):
                                    pso = ops2.tile([128, 1024], F32, tag="pso")
                                    for c in range(NHL):
                                        for rb in range(2):
                                            rsl = slice(
                                                rbh * 1024 + rb * 512,
                                                rbh * 1024 + (rb + 1) * 512,
                                            )
                                            nc.tensor.matmul(
                                                pso[:, rb * 512:(rb + 1) * 512],
                                                wo_strip[:, c, :],
                                                y_sbuf[:, c, rsl],
                                                start=(c == 0),
                                                stop=(c == NHL - 1),
                                            )
                                    for ob in range(2):
                                        ot = ooutp.tile([128, 512], F32, tag="ot")
                                        nc.scalar.copy(ot[:], pso[:, ob * 512:(ob + 1) * 512])
                                        nc.gpsimd.dma_start(
                                            out=outT.ap()[
                                                oc * 128:(oc + 1) * 128,
                                                rbh * 1024 + ob * 512:rbh * 1024 + (ob + 1) * 512,
                                            ],
                                            in_=ot[:],
                                        )

    nc.finalize()
    return nc


_NC_CACHE = None


def get_nc():
    global _NC_CACHE
    if _NC_CACHE is None:
        _NC_CACHE = _build_nc()
    return _NC_CACHE


def build_nrep(nrep):
    return _build_nc(nrep=nrep)


def _trig_tables(offset):
    inv_freq = 1.0 / (ROPE_BASE ** (np.arange(0, HD, 2, dtype=np.float64) / HD))
    pos = np.arange(offset, offset + T, dtype=np.float64)
    ang = pos[:, None] * inv_freq[None, :]        # [T, 64]
    cos = np.cos(ang)
    sin = np.sin(ang)
    cosT = np.concatenate([cos, cos], axis=1).T.astype(ml_dtypes.bfloat16)   # [128, T]
    sinT = np.concatenate([-sin, sin], axis=1).T.astype(ml_dtypes.bfloat16)  # sign-folded
    return np.ascontiguousarray(cosT), np.ascontiguousarray(sinT)


def _mask_table():
    """Additive mask [128, 1024]: two chunk blocks x (2 heads x 256 q).
    Block cb in {0,1} masks key chunk 2jj+cb against queries jj*256..+255:
    condition cb*128 + k <= j."""
    k = np.arange(128)[:, None]
    j = np.arange(256)[None, :]
    m = np.zeros((128, 1024), dtype=ml_dtypes.bfloat16)
    blk0 = np.where(k <= j, 0.0, MASK_NEG)          # [tril | zeros]
    blk1 = np.where(128 + k <= j, 0.0, MASK_NEG)    # [neg | tril]
    m[:, 0:256] = blk0
    m[:, 256:512] = blk0
    m[:, 512:768] = blk1
    m[:, 768:1024] = blk1
    return m


def _consts_table():
    c = np.zeros((128, 256), dtype=np.float32)
    c[:, 0:128] = 1.0
    # rotate-half swap matrix R (symmetric involution): R[d, d+64]=1, R[d+64, d]=1
    d = np.arange(64)
    c[d, 128 + d + 64] = 1.0
    c[d + 64, 128 + d] = 1.0
    return c


def make_in_maps(x, Wq, Wk, Wv, Wo, offset):
    x = np.asarray(x, dtype=np.float32)
    Wq = np.asarray(Wq, dtype=np.float32)
    Wk = np.asarray(Wk, dtype=np.float32)
    Wv = np.asarray(Wv, dtype=np.float32)
    Wo = np.asarray(Wo, dtype=np.float32)
    offset = int(np.asarray(offset))

    scale = 1.0 / math.sqrt(HD)
    wqT = np.ascontiguousarray((Wq * scale).T)     # [C, C]
    wkT = np.ascontiguousarray(Wk.T)               # [C, 512]
    wvT = np.ascontiguousarray(Wv.T)
    woT = np.ascontiguousarray(Wo.T)               # [C, C]
    cosT, sinT = _trig_tables(offset)
    mask = _mask_table()
    consts = _consts_table()

    in_maps = []
    for core in range(8):
        b, s = core // 2, core % 2
        xb = x[b]                                   # [T, C]
        in_maps.append({
            "xT": np.ascontiguousarray(xb.T).astype(ml_dtypes.bfloat16),
            "wqT": np.ascontiguousarray(wqT[:, s * CL:(s + 1) * CL]).astype(ml_dtypes.bfloat16),
            "wkT": np.ascontiguousarray(wkT[:, s * 256:(s + 1) * 256]).astype(ml_dtypes.bfloat16),
            "wvT": np.ascontiguousarray(wvT[:, s * 256:(s + 1) * 256]).astype(ml_dtypes.bfloat16),
            "woT": np.ascontiguousarray(woT[s * CL:(s + 1) * CL, :]).astype(ml_dtypes.bfloat16),
            "cosT": cosT, "sinT": sinT,
            "maskadd": mask,
            "consts": consts,
            "ones_bf": np.ones((128, 128), dtype=ml_dtypes.bfloat16),
        })
    return in_maps


def assemble_output(results):
    out = np.empty((B, T, C), dtype=np.float32)
    for b in range(B):
        acc = results[2 * b]["outT"] + results[2 * b + 1]["outT"]
        out[b] = acc.T
    return out


def kernel(x, Wq, Wk, Wv, Wo, offset):
    nc = get_nc()
    in_maps = make_in_maps(x, Wq, Wk, Wv, Wo, offset)
    res = run_bass_kernel_spmd(nc, in_maps, core_ids=list(range(8)))
    return assemble_output(res.results)
